# revision 1
# baseline (speedup 1.0000x reference)
"""DiscreteMMSE Trainium2 Bass kernel.

Math (per batch row b):
  Z = data[b] @ W                      [N, T]   (W = squeeze(task_pool).T)
  sq = (Z - targets[b][:, None])^2     [N, T]
  S[i] = sum_{n<i} sq[n]               (strict cumsum over N; S[0] = 0)
  E = exp(-0.5*(S - min_t S))          (softmax-stable weights)
  out[b, i] = targets[b, i] + (sum_t E[i]*(Z-t)[i]) / (sum_t E[i])

Identical to the reference softmax-posterior MMSE prediction: the Gaussian
log-pdf constant and row-wise shifts cancel in the softmax, and
pred = sum_t post*Z = t + sum_t post*(Z-t). Row 0 (uniform prior over
tasks) falls out of the strict cumsum.

Layout per NeuronCore (pure data parallel over B: 8 rows each, no
collectives). N=256 rows on partitions as two 128-row chunks; T=4096 on
the free dim. Engine budget per (b,jt) round is balanced across all four
compute engines:
  - TensorE: Z via ONE 128-contraction f32r matmul per 512-slice with
    lhsT=[d_hi; d_lo] (hi/lo split of data.T) and rhs=[W; W] (full fp32
    bits bitcast to f32r; the PE's 12-bit mantissa drop on read is
    compensated by the lo plane of data, and W's truncation noise is
    ~1e-3 per Z entry -- well within tolerance). Strict cumsum over N via
    triangular-ones f32r matmuls (chunk1 adds ones.T@sq0), reading the
    fp32 sq tile bitcast as f32r (truncation noise ~0.07 nats on logits).
    Z is recomputed for stage 2 (cheaper than buffering it in SBUF).
  - ScalarE (Act): sq = Square(Z + bias) with per-partition bias=-targets
    straight out of PSUM (written as f32r so the cumsum matmul input is
    legal); per-jt Exp with scale=-0.5, bias=0.5*min_t S and
    accum_out = denominator partials (Act pays a ~187ns accumulator-read
    per accum op -- still the cheapest den option); half the setup
    transpose evacuations.
  - VectorE (DVE): cumsum evacuations PSUM->SBUF fused with the running
    row-min (tensor_scalar accum min); numerator via ONE fused
    scalar_tensor_tensor: (Z_psum - t)*E with accum_out = running sum.
  - GpSimd (Pool): no PSUM port and no working accum ops, so it only
    duplicates W in setup.
  - modulo-scheduled emission: per-jt rounds interleave batch b's stage-1
    chain (Z->sq->cumsum->evac) with batch b-1's stage-2 chain
    (exp->Zre->E*(Z-t)); the den/num reductions are deferred two
    iterations so they never stall VectorE; the per-chunk exp bias is
    emitted as soon as that chunk's last row-min partial lands. The Tile
    scheduler is work-conserving per engine, so the emission order only
    sets priorities.
  - startup: W and data loads are chunked and ride separate DMA rings,
    and the W / data.T constants are split into per-jt / per-batch tiles
    so the first matmuls only depend on the first chunks (dependency
    tracking is tile-granular).
"""

import numpy as np

B, N, D, T = 64, 256, 64, 4096
NCORES = 8
BPC = B // NCORES  # batch rows per core
NCH = 2            # partition chunks of N
PB = 128           # partitions per chunk
PT = 1024          # psum tile free size (2 banks)
MT = 512           # matmul moving free size (1 bank)
NJT = T // PT      # psum tiles per chunk row
NMM = PT // MT     # matmuls per psum tile

_cached_nc = None


def _evac_on_act(b, jt, c):
    """Which cumsum evacuations run as ScalarE Copy + VectorE SBUF-min
    instead of the fused VectorE evac.

    Offloading looked good on paper (Act had slack) but every tested
    pattern - steady-state and final-batch-only - lost 0.3-5us to
    scheduling ripples, so none are offloaded.

    Note for future work: tensor_tensor_reduce with a chained AP
    accumulator passes the compiler and simulator but FAULTS the
    execution unit on real TRN2 (NRT_EXEC_UNIT_UNRECOVERABLE); the
    per-jt scalar_tensor_tensor + final reduce used here is the safe
    form."""
    return False


def _build():
    import concourse.bacc as bacc
    import concourse.mybir as mybir
    import concourse.tile as tile
    from concourse import masks

    F32 = mybir.dt.float32
    F32R = mybir.dt.float32r
    AF = mybir.ActivationFunctionType
    OP = mybir.AluOpType

    nc = bacc.Bacc("TRN2", debug=False)
    data_d = nc.dram_tensor("data", (BPC, N, D), F32, kind="ExternalInput")
    targ_d = nc.dram_tensor("targets", (BPC, N), F32, kind="ExternalInput")
    pool_d = nc.dram_tensor("task_pool", (T, D), F32, kind="ExternalInput")
    out_d = nc.dram_tensor("out", (BPC, N), F32, kind="ExternalOutput")

    with tile.TileContext(nc) as tc:
        with tc.tile_pool(name="const", bufs=1) as const:
            utri = const.tile([PB, PB], F32R)     # strictly-upper ones (lhsT)
            onesm = const.tile([PB, PB], F32R)    # all-ones
            # [W ; W] split per-jt so the first matmuls only wait on the
            # first quarter of the task-pool transposes
            wrept = [const.tile([PB, PT], F32R, name=f"wrept{j}", tag=f"wrept{j}")
                     for j in range(NJT)]
            # [data.T hi ; data.T lo] split per batch row for the same reason
            dstkb = [const.tile([PB, N], F32R, name=f"dstk{b}", tag=f"dstk{b}")
                     for b in range(BPC)]
            tpart = [const.tile([PB, BPC], F32, name=f"tpart{c}", tag=f"tpart{c}") for c in range(NCH)]
            tneg = [const.tile([PB, BPC], F32, name=f"tneg{c}", tag=f"tneg{c}") for c in range(NCH)]
            den = [const.tile([PB, BPC], F32, name=f"den{c}", tag=f"den{c}") for c in range(NCH)]
            num = [const.tile([PB, BPC], F32, name=f"num{c}", tag=f"num{c}") for c in range(NCH)]

            nc.any.memset(onesm[:].bitcast(F32), 1.0)

            # ---- setup: transpose task_pool and data into lhsT layouts ----
            with (
                tc.tile_pool(name="ld", bufs=1) as ld,
                tc.tile_pool(name="tps", bufs=6, space="PSUM") as tps,
            ):
                # kick off all input DMAs first so they overlap mask setup
                wbig = ld.tile([PB, (T // PB) * D], F32, tag="wbig", name="wbig")
                NK = T // PB
                KC = NK // 8  # chunk the load so transposes overlap the DMA
                # data + targets ride the Activation DMA ring so they overlap
                # the task-pool load on the SP ring
                for c in range(NCH):
                    nc.scalar.dma_start(
                        tpart[c][:],
                        targ_d[:, c * PB : (c + 1) * PB].rearrange("b p -> p b"),
                    )
                BH = BPC // 2
                dbh = [ld.tile([PB, BH * NCH * D], F32, tag=f"dbh{h}", name=f"dbh{h}")
                       for h in range(2)]
                for h in range(2):
                    nc.scalar.dma_start(
                        dbh[h][:].rearrange("p (b c d) -> p b c d", c=NCH, d=D),
                        data_d[h * BH : (h + 1) * BH].rearrange(
                            "b (c p) d -> p b c d", p=PB
                        ),
                    )
                for q in range(8):
                    nc.sync.dma_start(
                        wbig[:, q * KC * D : (q + 1) * KC * D].rearrange(
                            "p (k d) -> p k d", d=D
                        ),
                        pool_d[q * KC * PB : (q + 1) * KC * PB].rearrange(
                            "(k p) d -> p k d", p=PB
                        ),
                    )
                ident = ld.tile([PB, PB], F32, tag="ident", name="ident")
                masks.make_identity(nc, ident[:])
                utri_f = ld.tile([PB, PB], F32, tag="utri_f", name="utri_f")
                masks.make_upper_triangular(nc, utri_f[:], 1.0, diag=False)
                nc.vector.tensor_copy(utri[:], utri_f[:])
                KPJ = NK // NJT  # k-blocks per jt tile
                for k in range(NK):
                    j, kk = k // KPJ, k % KPJ
                    pt = tps.tile([D, PB], F32, tag="pt", name="pt")
                    nc.tensor.transpose(pt[:], wbig[:, k * D : (k + 1) * D], ident[:])
                    # split the PSUM->SBUF evacuations across Act and DVE
                    if k % 2 == 0:
                        nc.scalar.activation(
                            wrept[j][0:D, kk * PB : (kk + 1) * PB], pt[:], AF.Copy
                        )
                    else:
                        nc.vector.tensor_copy(
                            wrept[j][0:D, kk * PB : (kk + 1) * PB], pt[:]
                        )
                    if kk == KPJ - 1:
                        # duplicate the f32r-rounded W into the low 64
                        # partitions (GpSimd: SBUF-only, otherwise idle)
                        nc.gpsimd.tensor_copy(
                            wrept[j][D : 2 * D, :], wrept[j][0:D, :]
                        )
                for b in range(BPC):
                    for c in range(NCH):
                        cs = slice(c * PB, (c + 1) * PB)
                        pt = tps.tile([D, PB], F32, tag="pt", name="pt")
                        bb = b % (BPC // 2)
                        nc.tensor.transpose(
                            pt[:],
                            dbh[b // (BPC // 2)][
                                :, (bb * NCH + c) * D : (bb * NCH + c + 1) * D
                            ],
                            ident[:],
                        )
                        # hi: f32r-rounding convert copy; lo: exact fp32 rest
                        nc.scalar.activation(dstkb[b][0:D, cs], pt[:], AF.Copy)
                        nc.vector.tensor_sub(
                            dstkb[b][D : 2 * D, cs], pt[:],
                            dstkb[b][0:D, cs].bitcast(F32),
                        )
                for c in range(NCH):
                    nc.vector.tensor_scalar(
                        out=tneg[c][:], in0=tpart[c][:], scalar1=-1.0,
                        scalar2=None, op0=OP.mult,
                    )

            # ---- main pipeline ----
            with (
                tc.tile_pool(name="sqp", bufs=3) as sqp,
                tc.tile_pool(name="avp", bufs=2) as avp,
                tc.tile_pool(name="evp", bufs=2) as evp,
                tc.tile_pool(name="mscr", bufs=2) as mscrp,
                tc.tile_pool(name="small", bufs=4) as small,
                tc.tile_pool(name="rpp", bufs=2, space="PSUM") as rpp,
                tc.tile_pool(name="spp", bufs=1, space="PSUM") as spp,
                tc.tile_pool(name="rp2p", bufs=1, space="PSUM") as rp2p,
            ):

                def s1_alloc(b):
                    av = [
                        avp.tile([PB, T], F32, tag=f"av{c}", name=f"av{c}")
                        for c in range(NCH)
                    ]
                    mx2 = [
                        small.tile([PB, NJT], F32, tag=f"mx2{c}", name=f"mx2{c}")
                        for c in range(NCH)
                    ]
                    return av, mx2

                def _bias_emit(b, c, mx2):
                    """exp bias = 0.5 * min_t S; emitted per chunk as soon as
                    that chunk's last evac partial lands (shortens the
                    evac->bias->exp->numerator boundary chain)."""
                    scr = small.tile([PB, NJT], F32, tag=f"bsc{c}", name=f"bsc{c}")
                    bias = small.tile([PB, 1], F32, tag=f"bias{c}", name=f"bias{c}")
                    nc.vector.tensor_scalar(
                        out=scr[:], in0=mx2[c][:], scalar1=0.5, scalar2=None,
                        op0=OP.mult, op1=OP.min, accum_out=bias[:],
                    )
                    return bias

                def s1_round(b, jt, av, mx2, biases):
                    """per-jt chain: Z -> sq -> cumsum -> evac(+row min)."""
                    js = slice(jt * PT, (jt + 1) * PT)
                    sqs = []
                    for c in range(NCH):
                        cs = slice(c * PB, (c + 1) * PB)
                        rp = rpp.tile([PB, PT], F32, tag="rp", name="rp")
                        for h in range(NMM):
                            nc.tensor.matmul(
                                rp[:, h * MT : (h + 1) * MT],
                                dstkb[b][:, cs],
                                wrept[jt][:, h * MT : (h + 1) * MT],
                            )
                        sq = sqp.tile([PB, PT], F32R, tag=f"sq{c}", name=f"sq{c}")
                        nc.scalar.activation(
                            sq[:], rp[:], AF.Square,
                            bias=tneg[c][:, b : b + 1], scale=1.0,
                        )
                        sqs.append(sq)
                    for c in range(NCH):
                        # pipeline fill (b==0): no stage-2 exists yet, so the
                        # idle rp2p pool double-buffers the cumsum tiles --
                        # the mirror of the tail's rpp borrow
                        if b == 0 and c == 1:
                            sp = rp2p.tile([PB, PT], F32, tag="rp2", name="sp")
                        else:
                            sp = spp.tile([PB, PT], F32, tag="sp", name="sp")
                        for h in range(NMM):
                            hsl = slice(h * MT, (h + 1) * MT)
                            nc.tensor.matmul(
                                sp[:, hsl], utri[:], sqs[c][:, hsl],
                                start=True, stop=(c == 0),
                            )
                            if c == 1:
                                nc.tensor.matmul(
                                    sp[:, hsl], onesm[:], sqs[0][:, hsl],
                                    start=False, stop=True,
                                )
                        if _evac_on_act(b, jt, c):
                            # offload: Act copies PSUM->SBUF; the row min runs
                            # on VectorE from SBUF (all-SBUF TensorScalar gets
                            # the 2x DVE rate; TensorReduce does not)
                            nc.scalar.activation(av[c][:, js], sp[:], AF.Copy)
                            ms = mscrp.tile([PB, PT], F32, tag="ms", name="ms")
                            nc.vector.tensor_scalar(
                                out=ms[:], in0=av[c][:, js], scalar1=1.0,
                                scalar2=None, op0=OP.mult, op1=OP.min,
                                accum_out=mx2[c][:, jt : jt + 1],
                            )
                        else:
                            nc.vector.tensor_scalar(
                                out=av[c][:, js], in0=sp[:], scalar1=1.0,
                                scalar2=None, op0=OP.mult, op1=OP.min,
                                accum_out=mx2[c][:, jt : jt + 1],
                            )
                        if jt == NJT - 1:
                            biases.append(_bias_emit(b, c, mx2))

                def s2_alloc(b):
                    den4 = [
                        small.tile([PB, NJT], F32, tag=f"den4{c}", name=f"den4{c}")
                        for c in range(NCH)
                    ]
                    num4 = [
                        small.tile([PB, NJT], F32, tag=f"num4{c}", name=f"num4{c}")
                        for c in range(NCH)
                    ]
                    return den4, num4

                def s2_round(b, jt, av, biases, den4, num4, tail=False):
                    """exp (accum den) -> Z recompute -> fused (Z-t)*E.

                    In the drain (tail=True) there is no stage-1 work, so the
                    Z recompute borrows the idle double-buffered rpp pool:
                    with the single-buffer rp2p, each numerator would wait
                    ~0.7us for the previous one to free the PSUM slot."""
                    js = slice(jt * PT, (jt + 1) * PT)
                    for c in range(NCH):
                        cs = slice(c * PB, (c + 1) * PB)
                        ev = evp.tile([PB, PT], F32, tag=f"E{c}", name=f"E{c}")
                        nc.scalar.activation(
                            ev[:], av[c][:, js], AF.Exp,
                            bias=biases[c][:], scale=-0.5,
                            accum_out=den4[c][:, jt : jt + 1],
                        )
                        evs = ev[:]
                        pool = rpp if tail else rp2p
                        rp2 = pool.tile([PB, PT], F32, tag="rp" if tail else "rp2", name="rp2")
                        for h in range(NMM):
                            nc.tensor.matmul(
                                rp2[:, h * MT : (h + 1) * MT],
                                dstkb[b][:, cs],
                                wrept[jt][:, h * MT : (h + 1) * MT],
                            )
                        ns = mscrp.tile([PB, PT], F32, tag="ns", name="ns")
                        nc.vector.scalar_tensor_tensor(
                            out=ns[:], in0=rp2[:],
                            scalar=tpart[c][:, b : b + 1], in1=evs,
                            op0=OP.subtract, op1=OP.mult,
                            accum_out=num4[c][:, jt : jt + 1],
                        )

                def s2_finish(b, den4, num4):
                    for c in range(NCH):
                        nc.vector.tensor_reduce(
                            num[c][:, b : b + 1], num4[c][:],
                            axis=mybir.AxisListType.X, op=OP.add,
                        )
                        nc.vector.tensor_reduce(
                            den[c][:, b : b + 1], den4[c][:],
                            axis=mybir.AxisListType.X, op=OP.add,
                        )

                # modulo-scheduled pipeline: per-jt rounds interleave batch b's
                # stage-1 chain with batch b-1's stage-2 chain so each engine's
                # in-order stream always has ready work at the front.
                # s2_finish(b) is deliberately emitted one iteration later
                # (mid-round): den/num are only read by the finals, and
                # emitting the reduces right after the last round would stall
                # VectorE on the exp->den chain of the final jt tile.
                prev = None
                fin = None
                for b in range(BPC):
                    av, mx2 = s1_alloc(b)
                    biases = []
                    if prev is not None:
                        pb, pav, pbias, pden4, pnum4 = prev
                    for jt in range(NJT):
                        # s1 first: at batch boundaries the next batch's
                        # squares must outrank the previous batch's exps on
                        # ScalarE or VectorE starves waiting for cumsums
                        s1_round(b, jt, av, mx2, biases)
                        if prev is not None:
                            s2_round(pb, jt, pav, pbias, pden4, pnum4)
                        if jt == 2 and fin is not None:
                            s2_finish(*fin)
                            fin = None
                    if prev is not None:
                        fin = (pb, pden4, pnum4)
                    den4, num4 = s2_alloc(b)
                    prev = (b, av, biases, den4, num4)
                pb, pav, pbias, pden4, pnum4 = prev
                for jt in range(NJT):
                    s2_round(pb, jt, pav, pbias, pden4, pnum4, tail=True)
                    if jt == 2 and fin is not None:
                        s2_finish(*fin)
                        fin = None
                s2_finish(pb, pden4, pnum4)

                # finals: out = targets + num/den
                for c in range(NCH):
                    rec = small.tile([PB, BPC], F32, tag=f"rec{c}", name=f"rec{c}")
                    prod = small.tile([PB, BPC], F32, tag=f"prod{c}", name=f"prod{c}")
                    outv = small.tile([PB, BPC], F32, tag=f"outv{c}", name=f"outv{c}")
                    nc.vector.reciprocal(rec[:], den[c][:])
                    nc.vector.tensor_mul(prod[:], num[c][:], rec[:])
                    nc.vector.tensor_add(outv[:], tpart[c][:], prod[:])
                    ov = out_d[:, c * PB : (c + 1) * PB].rearrange("b p -> p b")
                    nc.sync.dma_start(ov, outv[:])

    nc.compile()
    return nc


def _get_nc():
    global _cached_nc
    if _cached_nc is None:
        _cached_nc = _build()
    return _cached_nc


_cached_runner = None


def _get_runner():
    """Build once: a cached jax.jit shard_map over the 8 NeuronCores.

    run_bass_kernel_spmd/run_bass_via_pjrt construct a fresh jax.jit closure
    per call (full retrace); caching the callable keeps repeat calls cheap.
    """
    global _cached_runner
    if _cached_runner is None:
        import jax
        from jax.sharding import Mesh, PartitionSpec
        from concourse import bass2jax
        from concourse.bass2jax import _bass_exec_p, partition_id_tensor
        import concourse.mybir as mybir

        try:
            from jax.experimental.shard_map import shard_map
        except ImportError:
            from jax.shard_map import shard_map

        bass2jax.install_neuronx_cc_hook()
        nc = _get_nc()
        partition_name = (
            nc.partition_id_tensor.name if nc.partition_id_tensor else None
        )
        in_names, out_names, out_avals, zero_outs = [], [], [], []
        for alloc in nc.m.functions[0].allocations:
            if not isinstance(alloc, mybir.MemoryLocationSet):
                continue
            name = alloc.memorylocations[0].name
            if alloc.kind == "ExternalInput":
                if name != partition_name:
                    in_names.append(name)
            elif alloc.kind == "ExternalOutput":
                out_names.append(name)
                shape = tuple(alloc.tensor_shape)
                dtype = mybir.dt.np(alloc.dtype)
                out_avals.append(jax.core.ShapedArray(shape, dtype))
                zero_outs.append(np.zeros((NCORES * shape[0], *shape[1:]), dtype))
        n_params = len(in_names)
        all_names = list(in_names) + list(out_names)
        if partition_name is not None:
            all_names.append(partition_name)
        donate = tuple(range(n_params, n_params + len(out_names)))

        def _body(*args):
            operands = list(args)
            if partition_name is not None:
                operands.append(partition_id_tensor())
            return tuple(
                _bass_exec_p.bind(
                    *operands,
                    out_avals=tuple(out_avals),
                    in_names=tuple(all_names),
                    out_names=tuple(out_names),
                    lowering_input_output_aliases=(),
                    sim_require_finite=True,
                    sim_require_nnan=True,
                    nc=nc,
                )
            )

        devices = jax.devices()[:NCORES]
        mesh = Mesh(np.asarray(devices), ("core",))
        in_specs = tuple(
            PartitionSpec() if name == "task_pool" else PartitionSpec("core")
            for name in in_names
        ) + (PartitionSpec("core"),) * len(out_names)
        sharded = jax.jit(
            shard_map(
                _body,
                mesh=mesh,
                in_specs=in_specs,
                out_specs=(PartitionSpec("core"),) * len(out_names),
                check_rep=False,
            ),
            donate_argnums=donate,
            keep_unused=True,
        )
        _cached_runner = (sharded, in_names, out_names, out_avals, zero_outs)
    return _cached_runner


def _kernel_fallback(data, targets, tp):
    """Robust path via the stock SPMD runner (fresh jit each call)."""
    from concourse.bass_utils import run_bass_kernel_spmd

    nc = _get_nc()
    in_maps = [
        {
            "data": data[i * BPC : (i + 1) * BPC],
            "targets": targets[i * BPC : (i + 1) * BPC],
            "task_pool": tp,
        }
        for i in range(NCORES)
    ]
    res = run_bass_kernel_spmd(nc, in_maps, core_ids=list(range(NCORES)))
    return np.concatenate([r["out"] for r in res.results], axis=0)


def kernel(data, targets, task_pool, **_):
    data = np.ascontiguousarray(np.asarray(data, np.float32))
    targets = np.ascontiguousarray(np.asarray(targets, np.float32))
    tp = np.ascontiguousarray(np.asarray(task_pool, np.float32).reshape(T, D))

    try:
        sharded, in_names, out_names, out_avals, zero_outs = _get_runner()
        full = {
            "data": data.reshape(NCORES * BPC, N, D),
            "targets": targets.reshape(NCORES * BPC, N),
            "task_pool": tp,
        }
        args = [full[name] for name in in_names]
        args += [np.zeros_like(z) for z in zero_outs]
        outs = sharded(*args)
        out = np.asarray(outs[out_names.index("out")])
        return out.reshape(B, N)
    except Exception:
        return _kernel_fallback(data, targets, tp)



# revision 8
# speedup vs baseline: 1.0535x; 1.0535x over previous
"""DiscreteMMSE Trainium2 Bass kernel.

Math (per batch row b):
  Z = data[b] @ W                          [N, T]  (W = squeeze(task_pool).T)
  sq = (Z - targets[b][:, None])^2         [N, T]
  S[i] = sum_{n<i} sq[n]                   (strict cumsum over N; S[0] = 0)
  E = exp(-0.5*(S - min_t S))              (softmax-stable weights)
  out[b, i] = (data[b, i] . sum_t E[i,t] w_t) / (sum_t E[i,t])

Identical to the reference softmax-posterior MMSE prediction: the Gaussian
log-pdf constant and row-wise shifts cancel in the softmax, and
pred = sum_t post*Z = data . (sum_t post*w_t). Row 0 (uniform prior over
tasks) falls out of the strict cumsum (S[0]=0 -> E=1 -> uniform).

Layout per NeuronCore (pure data parallel over B: 8 rows each, no
collectives). Stage 1 runs with N=256 on partitions (two 128-row chunks)
and T=4096 on the free dim; stage 2 flips to T on partitions via a DMA
transpose so the posterior-weighted sums become TensorE matmuls instead
of Vector/Scalar elementwise passes (the baseline's bottleneck: DVE 90%,
Act 85% busy vs PE 56%).

  - TensorE: Z via ONE 128-contraction f32r matmul per 512-slice with
    lhsT=[d_hi; d_lo] (hi/lo split of data.T) and rhs=[W; W]. Strict
    cumsum over N via triangular-ones f32r matmuls reading sq bitcast as
    f32r. Stage 2: wm[d,i] = sum_t E[t,i] W[d,t] plus a den row, as 32
    bf16 accumulating matmuls per batch (lhsT = task_pool rows + ones
    column, rhs = transposed E); final pred/den rows collected with
    per-batch selector matmuls into one [16, 256] PSUM tile.
  - ScalarE (Act): Exp with scale=-0.5, bias=0.5*min_t S, bf16 output
    (no accumulator read -- den comes from the matmul); a tunable share
    of the sq = Square(Z - targ) passes (bias = -targ).
  - VectorE (DVE): cumsum evacuations PSUM->SBUF fused with the running
    row-min (tensor_scalar accum min); the rest of the sq passes as a
    custom DVE op sq(Src0 - C0) (PSUM x PSUM tensor_tensor and pow are
    ISA-illegal; the custom op squares off a single PSUM read); one small
    [65, 256] wm*dataT product per batch.
  - DMA: per-(chunk, jt) 16-bit DMA transposes of E ([128, 1024] bf16 ->
    [128t, 8k, 128n] slices), 14ns per 16x128 xbar tile -- the engines
    never touch the transpose. Issued on the SP ring so Act's sequencer
    stays clear.
  - GpSimd (Pool): SBUF-only setup copies (W -> bf16 task-pool layout,
    dataT hi -> the [65, 256] stage-2 operand, wrept lo duplication).
  - modulo-scheduled emission: per-jt rounds interleave batch b's stage-1
    chain with batch b-1's stage-2 chain (Exp + transposes); the wmmse
    matmul group for jt is emitted one round later so its transposes have
    a full round of slack; sq passes go 8/0 Act/DVE on the fill batch
    (no stage-2 Exps yet) and 5/3 in steady state.
  - PSUM: one merged 3-buf pool cycles R and cumsum tiles (R's bank is
    dead once sq reads it), 1 bank for wm, 1 persistent bank for the
    pred/den accumulator: exactly 8 banks.

Note for future work: tensor_tensor_reduce with a chained AP accumulator
passes the compiler and simulator but FAULTS the execution unit on real
TRN2 (NRT_EXEC_UNIT_UNRECOVERABLE); per-tile accum + final reduce is the
safe form (not needed in this version -- den rides the matmul).
"""

import numpy as np

B, N, D, T = 64, 256, 64, 4096
NCORES = 8
BPC = B // NCORES  # batch rows per core
NCH = 2            # partition chunks of N
PB = 128           # partitions per chunk
PT = 1024          # psum tile free size (2 banks)
MT = 512           # matmul moving free size (1 bank)
NJT = T // PT      # psum tiles per chunk row
NMM = PT // MT     # matmuls per psum tile
NK = T // PB       # task-pool partition chunks (32)
KPJ = NK // NJT    # k-blocks per jt tile (8)

_cached_nc = None


def _register_sub_square():
    """Register a custom DVE op: out = (in0 - s0)^2, s0 per-partition.

    The stock ALU path cannot square off a single PSUM read (pow fails
    codegen, PSUM x PSUM tensor_tensor is ISA-illegal), so this uses the
    documented custom-DVE extension point (dve_ops.OPS) with a two-stage
    body sq(Src0 - C0). The uops sha is computed at registration the same
    way DveOp.compile() checks it."""
    from concourse import dve_ops
    from concourse.dve_spec import C0, Spec, Src0, _has_src1, lower, sq
    from concourse.dve_table_gen import dve_ver_for
    from concourse.dve_uop import DveOpSpec

    NAME = "SUB_SQUARE_ANT"
    for o in dve_ops.OPS:
        if o.name == NAME:
            return o
    spec = Spec(
        body=sq(Src0 - C0),
        reference=lambda in0, in1, s0, s1, imm2: (in0.astype(np.float32) - s0)
        ** 2,
    )
    row = max(dve_ops._SUB_OPCODE_FOR_NAME.values()) + 1
    assert row < 0x20, "custom-DVE opcode rows exhausted"
    ver = dve_ver_for("TRN2")
    sha = DveOpSpec(
        name=NAME, opcode=row, uops=lower(spec, ver=ver), rd1_en=_has_src1(spec)
    ).sha(ver)
    op = dve_ops.DveOp(NAME, spec, subdim=False, uops_sha={ver: sha})
    dve_ops._SUB_OPCODE_FOR_NAME[NAME] = row
    dve_ops.OPS.append(op)
    dve_ops.CUSTOM_DVE_SPECS[NAME] = spec
    return op


def _sq_on_act(b, jt, c):
    """Which sq = R^2 passes run on ScalarE (Act Square) vs VectorE
    (tensor_tensor R*R). Fill batch (no stage-2 Exps yet): all on Act.
    Steady state: 5 of 8 per batch on Act, 3 on DVE."""
    return True  # BISECT: custom-DVE sub-square suspected of HW fault


def _build():
    import concourse.bacc as bacc
    import concourse.mybir as mybir
    import concourse.tile as tile
    from concourse import masks

    F32 = mybir.dt.float32
    F32R = mybir.dt.float32r
    BF16 = mybir.dt.bfloat16
    AF = mybir.ActivationFunctionType
    OP = mybir.AluOpType

    subsq = _register_sub_square()
    nc = bacc.Bacc("TRN2", debug=False)
    data_d = nc.dram_tensor("data", (BPC, N, D), F32, kind="ExternalInput")
    targ_d = nc.dram_tensor("targets", (BPC, N), F32, kind="ExternalInput")
    pool_d = nc.dram_tensor("task_pool", (T, D), F32, kind="ExternalInput")
    out_d = nc.dram_tensor("out", (BPC, N), F32, kind="ExternalOutput")

    with tile.TileContext(nc) as tc:
        with tc.tile_pool(name="const", bufs=1) as const:
            utri = const.tile([PB, PB], F32R)     # strictly-upper ones (lhsT)
            onesm = const.tile([PB, PB], F32R)    # all-ones
            # [W ; W[0:63] ; -1] split per-jt so the first matmuls only wait
            # on the first quarter of the task-pool transposes
            wrept = [const.tile([PB, PT], F32R, name=f"wrept{j}", tag=f"wrept{j}")
                     for j in range(NJT)]
            # [data.T hi ; data.T lo[0:63] ; targets[b]] per batch row
            dstkb = [const.tile([PB, N], F32R, name=f"dstk{b}", tag=f"dstk{b}")
                     for b in range(BPC)]
            tpart = [const.tile([PB, BPC], F32, name=f"tpart{c}", tag=f"tpart{c}")
                     for c in range(NCH)]
            tneg = [const.tile([PB, BPC], F32, name=f"tneg{c}", tag=f"tneg{c}")
                    for c in range(NCH)]
            # stage-2 elementwise operand: [data.T hi ; ones] on 65 partitions
            dst65 = [const.tile([D + 1, N], F32, name=f"dst65_{b}", tag=f"dst65_{b}")
                     for b in range(BPC)]
            # task pool in native [t-part, d] layout, bf16, + ones column
            wcol = const.tile([PB, NK * (D + 1)], BF16, name="wcol", tag="wcol")
            # per-batch selector lhsT for the final pred/den reduction
            # (pred -> res row b, den -> res row 32+b: engine reads of the
            # den block must start at a 0/32/64/96 partition)
            redsel = const.tile([D + 1, 40 * BPC], F32R, name="redsel", tag="redsel")

            nc.any.memset(onesm[:].bitcast(F32), 1.0)
            nc.any.memset(redsel[:].bitcast(F32), 0.0)
            for b in range(BPC):
                # out row b <- sum_d prod[d] (pred), row 32+b <- prod[64] (den)
                nc.any.memset(
                    redsel[0:D, 40 * b + b : 40 * b + b + 1].bitcast(F32), 1.0
                )
                nc.any.memset(
                    redsel[D : D + 1, 40 * b + 32 + b : 40 * b + 33 + b].bitcast(F32),
                    1.0,
                )

            # ---- setup: transpose task_pool and data into lhsT layouts ----
            with (
                tc.tile_pool(name="ld", bufs=1) as ld,
                tc.tile_pool(name="tps", bufs=6, space="PSUM") as tps,
            ):
                # kick off all input DMAs first so they overlap mask setup
                wbig = ld.tile([PB, NK * D], F32, tag="wbig", name="wbig")
                KC = NK // 8  # chunk the load so transposes overlap the DMA
                # data + targets ride the Activation DMA ring so they overlap
                # the task-pool load on the SP ring
                for c in range(NCH):
                    nc.scalar.dma_start(
                        tpart[c][:],
                        targ_d[:, c * PB : (c + 1) * PB].rearrange("b p -> p b"),
                    )
                BH = BPC // 2
                dbh = [ld.tile([PB, BH * NCH * D], F32, tag=f"dbh{h}", name=f"dbh{h}")
                       for h in range(2)]
                for h in range(2):
                    nc.scalar.dma_start(
                        dbh[h][:].rearrange("p (b c d) -> p b c d", c=NCH, d=D),
                        data_d[h * BH : (h + 1) * BH].rearrange(
                            "b (c p) d -> p b c d", p=PB
                        ),
                    )
                for q in range(8):
                    nc.sync.dma_start(
                        wbig[:, q * KC * D : (q + 1) * KC * D].rearrange(
                            "p (k d) -> p k d", d=D
                        ),
                        pool_d[q * KC * PB : (q + 1) * KC * PB].rearrange(
                            "(k p) d -> p k d", p=PB
                        ),
                    )
                ident = ld.tile([PB, PB], F32, tag="ident", name="ident")
                masks.make_identity(nc, ident[:])
                utri_f = ld.tile([PB, PB], F32, tag="utri_f", name="utri_f")
                masks.make_upper_triangular(nc, utri_f[:], 1.0, diag=False)
                nc.vector.tensor_copy(utri[:], utri_f[:])
                for k in range(NK):
                    j, kk = k // KPJ, k % KPJ
                    pt = tps.tile([D, PB], F32, tag="pt", name="pt")
                    nc.tensor.transpose(pt[:], wbig[:, k * D : (k + 1) * D], ident[:])
                    # split the PSUM->SBUF evacuations across Act and DVE
                    if k % 2 == 0:
                        nc.scalar.activation(
                            wrept[j][0:D, kk * PB : (kk + 1) * PB], pt[:], AF.Copy
                        )
                    else:
                        nc.vector.tensor_copy(
                            wrept[j][0:D, kk * PB : (kk + 1) * PB], pt[:]
                        )
                    if kk == KPJ - 1:
                        # duplicate the f32r-rounded W into the low 64
                        # partitions (GpSimd: SBUF-only, otherwise idle)
                        nc.gpsimd.tensor_copy(
                            wrept[j][D : 2 * D, :], wrept[j][0:D, :]
                        )
                # task pool in native layout, bf16 + ones column (idle Pool)
                nc.gpsimd.tensor_copy(
                    wcol[:].rearrange("p (k e) -> p k e", e=D + 1)[:, :, 0:D],
                    wbig[:].rearrange("p (k d) -> p k d", d=D),
                )
                nc.any.memset(
                    wcol[:].rearrange("p (k e) -> p k e", e=D + 1)[:, :, D : D + 1],
                    1.0,
                )
                for b in range(BPC):
                    for c in range(NCH):
                        cs = slice(c * PB, (c + 1) * PB)
                        pt = tps.tile([D, PB], F32, tag="pt", name="pt")
                        bb = b % (BPC // 2)
                        nc.tensor.transpose(
                            pt[:],
                            dbh[b // (BPC // 2)][
                                :, (bb * NCH + c) * D : (bb * NCH + c + 1) * D
                            ],
                            ident[:],
                        )
                        # hi: f32r-rounding convert copy; lo: exact fp32 rest
                        nc.scalar.activation(dstkb[b][0:D, cs], pt[:], AF.Copy)
                        nc.vector.tensor_sub(
                            dstkb[b][D : 2 * D, cs], pt[:],
                            dstkb[b][0:D, cs].bitcast(F32),
                        )
                        # stage-2 operand rows (idle Pool)
                        nc.gpsimd.tensor_copy(
                            dst65[b][0:D, cs], dstkb[b][0:D, cs].bitcast(F32)
                        )
                for b in range(BPC):
                    nc.any.memset(dst65[b][D : D + 1, :], 1.0)
                for c in range(NCH):
                    nc.vector.tensor_scalar(
                        out=tneg[c][:], in0=tpart[c][:], scalar1=-1.0,
                        scalar2=None, op0=OP.mult,
                    )

            # ---- main pipeline ----
            with (
                tc.tile_pool(name="sqp", bufs=3) as sqp,
                tc.tile_pool(name="avp", bufs=2) as avp,
                tc.tile_pool(name="ebp", bufs=3) as ebp,
                tc.tile_pool(name="xp", bufs=2) as xp,
                tc.tile_pool(name="psp", bufs=2) as psp,
                tc.tile_pool(name="small", bufs=4) as small,
                tc.tile_pool(name="rsp", bufs=3, space="PSUM") as rsp,
                tc.tile_pool(name="wmp", bufs=1, space="PSUM") as wmp,
                tc.tile_pool(name="resp", bufs=1, space="PSUM") as resp,
            ):
                res = resp.tile([40, N], F32, name="res", tag="res")

                def s1_alloc(b):
                    av = [
                        avp.tile([PB, T], F32, tag=f"av{c}", name=f"av{c}")
                        for c in range(NCH)
                    ]
                    mx2 = [
                        small.tile([PB, NJT], F32, tag=f"mx2{c}", name=f"mx2{c}")
                        for c in range(NCH)
                    ]
                    return av, mx2

                def _bias_emit(b, c, mx2):
                    """exp bias = 0.5 * min_t S; emitted per chunk as soon as
                    that chunk's last evac partial lands."""
                    scr = small.tile([PB, NJT], F32, tag=f"bsc{c}", name=f"bsc{c}")
                    bias = small.tile([PB, 1], F32, tag=f"bias{c}", name=f"bias{c}")
                    nc.vector.tensor_scalar(
                        out=scr[:], in0=mx2[c][:], scalar1=0.5, scalar2=None,
                        op0=OP.mult, op1=OP.min, accum_out=bias[:],
                    )
                    return bias

                def s1_round(b, jt, av, mx2, biases):
                    """per-jt chain: R -> sq -> cumsum -> evac(+row min)."""
                    js = slice(jt * PT, (jt + 1) * PT)
                    sqs = []
                    for c in range(NCH):
                        cs = slice(c * PB, (c + 1) * PB)
                        rp = rsp.tile([PB, PT], F32, tag="rs", name="rs")
                        for h in range(NMM):
                            nc.tensor.matmul(
                                rp[:, h * MT : (h + 1) * MT],
                                dstkb[b][:, cs],
                                wrept[jt][:, h * MT : (h + 1) * MT],
                            )
                        sq = sqp.tile([PB, PT], F32R, tag=f"sq{c}", name=f"sq{c}")
                        if _sq_on_act(b, jt, c):
                            nc.scalar.activation(
                                sq[:], rp[:], AF.Square,
                                bias=tneg[c][:, b : b + 1], scale=1.0,
                            )
                        else:
                            # (Z - targ)^2 in one DVE op, single PSUM read
                            nc.vector._custom_dve(
                                subsq, out=sq[:], in0=rp[:],
                                s0=tpart[c][:, b : b + 1],
                            )
                        sqs.append(sq)
                    for c in range(NCH):
                        sp = rsp.tile([PB, PT], F32, tag="rs", name="rs")
                        for h in range(NMM):
                            hsl = slice(h * MT, (h + 1) * MT)
                            nc.tensor.matmul(
                                sp[:, hsl], utri[:], sqs[c][:, hsl],
                                start=True, stop=(c == 0),
                            )
                            if c == 1:
                                nc.tensor.matmul(
                                    sp[:, hsl], onesm[:], sqs[0][:, hsl],
                                    start=False, stop=True,
                                )
                        nc.vector.tensor_scalar(
                            out=av[c][:, js], in0=sp[:], scalar1=1.0,
                            scalar2=None, op0=OP.mult, op1=OP.min,
                            accum_out=mx2[c][:, jt : jt + 1],
                        )
                        if jt == NJT - 1:
                            biases.append(_bias_emit(b, c, mx2))

                def s2_alloc(b):
                    # transposed-E tiles: X[jt][tp, k_sub, n] = E[n, jt*1024 +
                    # k_sub*128 + tp]
                    return [
                        xp.tile([PB, KPJ * N], BF16, tag=f"x{j}", name=f"x{j}")
                        for j in range(NJT)
                    ]

                def s2_round(b, jt, av, biases, xts):
                    """exp (bf16) -> DMA transpose into the [t, n] layout."""
                    js = slice(jt * PT, (jt + 1) * PT)
                    for c in range(NCH):
                        ev = ebp.tile([PB, PT], BF16, tag=f"E{c}", name=f"E{c}")
                        nc.scalar.activation(
                            ev[:], av[c][:, js], AF.Exp,
                            bias=biases[c][:], scale=-0.5,
                        )
                        nc.sync.dma_start(
                            xts[jt][:]
                            .rearrange("p (k n) -> p k n", n=N)[
                                :, :, c * PB : (c + 1) * PB
                            ],
                            ev[:],
                            transpose=True,
                        )

                def s2_wmmse(b, jt, xts, wm):
                    """wm[d, i] += sum_t W[d, t] E[t, i] for jt's k-blocks;
                    row 64 accumulates den."""
                    for kk in range(KPJ):
                        k = jt * KPJ + kk
                        nc.tensor.matmul(
                            wm[:, :],
                            wcol[:].rearrange("p (k e) -> p k e", e=D + 1)[:, k, :],
                            xts[jt][:, kk * N : (kk + 1) * N],
                            start=(k == 0), stop=(k == NK - 1),
                        )

                def s2_finish(b, wm):
                    """prod = wm * [dataT; ones]; selector matmul accumulates
                    pred into res row b and den into row 8+b."""
                    ps = psp.tile([D + 1, N], F32R, tag="ps", name="ps")
                    nc.vector.tensor_mul(ps[:], wm[:], dst65[b][:])
                    nc.tensor.matmul(
                        res[:], redsel[:, 40 * b : 40 * (b + 1)], ps[:],
                        start=(b == 0), stop=(b == BPC - 1),
                    )

                # modulo-scheduled pipeline: per-jt rounds interleave batch b's
                # stage-1 chain with batch b-1's stage-2 chain. The wmmse
                # matmul group for jt is emitted one round later so PE's
                # in-order stream doesn't park on the transpose DMAs.
                prev = None
                wm = None
                for b in range(BPC):
                    av, mx2 = s1_alloc(b)
                    biases = []
                    if prev is not None:
                        pb, pav, pbias, pxts = prev
                    for jt in range(NJT):
                        # s1 first: the next batch's squares must outrank the
                        # previous batch's exps on ScalarE or VectorE starves
                        # waiting for cumsums
                        s1_round(b, jt, av, mx2, biases)
                        if prev is not None:
                            s2_round(pb, jt, pav, pbias, pxts)
                            if jt > 0:
                                s2_wmmse(pb, jt - 1, pxts, wm)
                    if prev is not None:
                        s2_wmmse(pb, NJT - 1, pxts, wm)
                        s2_finish(pb, wm)
                    xts = s2_alloc(b)
                    wm = wmp.tile([D + 1, N], F32, tag="wm", name="wm")
                    prev = (b, av, biases, xts)
                pb, pav, pbias, pxts = prev
                for jt in range(NJT):
                    s2_round(pb, jt, pav, pbias, pxts)
                    if jt > 0:
                        s2_wmmse(pb, jt - 1, pxts, wm)
                s2_wmmse(pb, NJT - 1, pxts, wm)
                s2_finish(pb, wm)

                # finals: out[b, i] = res[b, i] / res[32+b, i]
                resS = small.tile([40, N], F32, tag="resS", name="resS")
                rec = small.tile([BPC, N], F32, tag="rec", name="rec")
                outv = small.tile([BPC, N], F32, tag="outv", name="outv")
                nc.vector.tensor_copy(resS[:], res[:])
                nc.vector.reciprocal(rec[:], resS[32 : 32 + BPC, :])
                nc.vector.tensor_mul(outv[:], resS[0:BPC, :], rec[:])
                nc.sync.dma_start(out_d[:, :], outv[:])

    nc.compile()
    return nc


def _get_nc():
    global _cached_nc
    if _cached_nc is None:
        _cached_nc = _build()
    return _cached_nc


_cached_runner = None


def _get_runner():
    """Build once: a cached jax.jit shard_map over the 8 NeuronCores.

    run_bass_kernel_spmd/run_bass_via_pjrt construct a fresh jax.jit closure
    per call (full retrace); caching the callable keeps repeat calls cheap.
    """
    global _cached_runner
    if _cached_runner is None:
        import jax
        from jax.sharding import Mesh, PartitionSpec
        from concourse import bass2jax
        from concourse.bass2jax import _bass_exec_p, partition_id_tensor
        import concourse.mybir as mybir

        try:
            from jax.experimental.shard_map import shard_map
        except ImportError:
            from jax.shard_map import shard_map

        bass2jax.install_neuronx_cc_hook()
        nc = _get_nc()
        partition_name = (
            nc.partition_id_tensor.name if nc.partition_id_tensor else None
        )
        in_names, out_names, out_avals, zero_outs = [], [], [], []
        for alloc in nc.m.functions[0].allocations:
            if not isinstance(alloc, mybir.MemoryLocationSet):
                continue
            name = alloc.memorylocations[0].name
            if alloc.kind == "ExternalInput":
                if name != partition_name:
                    in_names.append(name)
            elif alloc.kind == "ExternalOutput":
                out_names.append(name)
                shape = tuple(alloc.tensor_shape)
                dtype = mybir.dt.np(alloc.dtype)
                out_avals.append(jax.core.ShapedArray(shape, dtype))
                zero_outs.append(np.zeros((NCORES * shape[0], *shape[1:]), dtype))
        n_params = len(in_names)
        all_names = list(in_names) + list(out_names)
        if partition_name is not None:
            all_names.append(partition_name)
        donate = tuple(range(n_params, n_params + len(out_names)))

        def _body(*args):
            operands = list(args)
            if partition_name is not None:
                operands.append(partition_id_tensor())
            return tuple(
                _bass_exec_p.bind(
                    *operands,
                    out_avals=tuple(out_avals),
                    in_names=tuple(all_names),
                    out_names=tuple(out_names),
                    lowering_input_output_aliases=(),
                    sim_require_finite=True,
                    sim_require_nnan=True,
                    nc=nc,
                )
            )

        devices = jax.devices()[:NCORES]
        mesh = Mesh(np.asarray(devices), ("core",))
        in_specs = tuple(
            PartitionSpec() if name == "task_pool" else PartitionSpec("core")
            for name in in_names
        ) + (PartitionSpec("core"),) * len(out_names)
        sharded = jax.jit(
            shard_map(
                _body,
                mesh=mesh,
                in_specs=in_specs,
                out_specs=(PartitionSpec("core"),) * len(out_names),
                check_rep=False,
            ),
            donate_argnums=donate,
            keep_unused=True,
        )
        _cached_runner = (sharded, in_names, out_names, out_avals, zero_outs)
    return _cached_runner


def _kernel_fallback(data, targets, tp):
    """Robust path via the stock SPMD runner (fresh jit each call)."""
    from concourse.bass_utils import run_bass_kernel_spmd

    nc = _get_nc()
    in_maps = [
        {
            "data": data[i * BPC : (i + 1) * BPC],
            "targets": targets[i * BPC : (i + 1) * BPC],
            "task_pool": tp,
        }
        for i in range(NCORES)
    ]
    res = run_bass_kernel_spmd(nc, in_maps, core_ids=list(range(NCORES)))
    return np.concatenate([r["out"] for r in res.results], axis=0)


def kernel(data, targets, task_pool, **_):
    data = np.ascontiguousarray(np.asarray(data, np.float32))
    targets = np.ascontiguousarray(np.asarray(targets, np.float32))
    tp = np.ascontiguousarray(np.asarray(task_pool, np.float32).reshape(T, D))

    try:
        sharded, in_names, out_names, out_avals, zero_outs = _get_runner()
        full = {
            "data": data.reshape(NCORES * BPC, N, D),
            "targets": targets.reshape(NCORES * BPC, N),
            "task_pool": tp,
        }
        args = [full[name] for name in in_names]
        args += [np.zeros_like(z) for z in zero_outs]
        outs = sharded(*args)
        out = np.asarray(outs[out_names.index("out")])
        return out.reshape(B, N)
    except Exception:
        return _kernel_fallback(data, targets, tp)
